# revision 57
# baseline (speedup 1.0000x reference)
"""Trainium2 Bass kernel for a GPT-style transformer block.

Shapes: x [2, 2048, 1024], H=16 heads, D=64, MLP 4x.

Distribution over 8 NeuronCores: data-parallel over batch (cores 0-3 ->
batch 0, cores 4-7 -> batch 1) x sequence-parallel over tokens inside
each batch group. Tokens are stride-4 interleaved (core s of the group
owns global tokens s, s+4, ...), which makes the causal-attention loop
structure identical on every core (required: all 8 cores share one SPMD
program); the rank-dependent causal diagonal masks are shipped as
per-core input data. The only collectives are two 4-rank AllGathers per
group (K first, then V, so the Q projection overlaps them).

LayerNorm gains/biases are folded into the adjacent weight matrices on
the host, so on-chip LN is a plain standardization with per-partition
(per-token) scalars. Matmuls run in bf16 (fp32 matmul is 4x slower on
PE; fp8 was measured to blow the 2e-2 error budget); accumulation, LN
statistics, softmax and residuals stay fp32. The softmax denominator
comes for free from an extra ones-column appended to V (row 64 of the
PV accumulator), so no partition-axis reduction is needed.

Scheduling notes (measured via the TimelineSim cost model + HW A/B):
- Attention is ACT-bound (~91us of exp per core). The QK -> exp ->
  mask -> PV chain is software-pipelined with a 2-tile lookahead (PE is
  in-order; a PV waiting on exp would stall later QKs queued behind it),
  psS bufs=3. The causal mask is one fused DVE multiply per head pair;
  the softmax normalizer broadcast runs on the idle Pool engine.
- Both AllGathers are split in halves and launched as soon as their
  half of K/V is projected; readback is ordered by first consumer
  (K-half0, V-half0, V-half1, K-half1).
- Biases enter PSUM as the FIRST accumulation (ones x bias matmul) so
  no serial bias matmul sits in any dependency tail; the last 4 Wproj
  chunks run block-major so output stores overlap remaining matmuls.
- LEVEL<5 code paths are phase-bisection scaffolding for measurements
  only; the graded path is LEVEL=5, REPS=1.
"""

import os
import sys

for _p in ("/opt/trn_rl_repo", "/root/.axon_site/_ro/trn_rl_repo"):
    if os.path.isdir(_p) and _p not in sys.path:
        sys.path.insert(0, _p)

import numpy as np
import ml_dtypes

import concourse.bass as bass
import concourse.bacc as bacc
import concourse.mybir as mybir
import concourse.tile as tile
from concourse.bass_utils import run_bass_kernel_spmd

F32 = mybir.dt.float32
F32R = mybir.dt.float32r
BF16 = mybir.dt.bfloat16
AF = mybir.ActivationFunctionType
OP = mybir.AluOpType
BF16_NP = ml_dtypes.bfloat16

B, T, C = 2, 2048, 1024
H, D = 16, 64
FF = 4 * C
EPS = 1e-5
P = 128
CH = C // P        # 8 chunks of the channel dim
NBQ = 4            # local 128-token blocks per core (512 tokens)
NR = 4             # seq ranks per batch group
FCH = FF // P      # 32 chunks of the FF dim
HP = H // 2        # head pairs

TRACE = False           # set by test harness for profiling
LEVEL = 5               # phase bisection: 1=QKV 2=+AG 3=+attn 4=+Wo 5=full
REPS = 1                # timing: emit the whole block N times, serialized via x
LAST_RESULTS = None     # BassKernelResults of the last run

_CACHE = {}


def _ln_stats(nc, pool, src, tag):
    """Phase 1 of LN: per-token sum and sum-of-squares of `src`.

    The plain sum runs on the (otherwise idle) Pool engine and the square
    sum on ACT, keeping DVE free for the finalize/normalize work.
    """
    s1 = pool.tile([P, 1], F32, name=f"ln_s1_{tag}", tag=f"ln_s1_{tag}")
    ssq = pool.tile([P, 1], F32, name=f"ln_ssq_{tag}", tag=f"ln_ssq_{tag}")
    sqs = pool.tile([P, C], BF16, name=f"ln_sqs_{tag}", tag="ln_sqs", bufs=1)
    nc.scalar.activation(sqs[:, :], src, AF.Copy, accum_out=s1[:, :])
    nc.scalar.activation(sqs[:, :], src, AF.Square, accum_out=ssq[:, :])
    return s1, ssq


def _ln_finalize(nc, pool, src, z_bf, s1, ssq, tag, s1b=None):
    """Phase 2 of LN: turn (sum, sumsq) into (x-mean)*rstd -> z_bf."""
    mean = pool.tile([P, 1], F32, name=f"ln_mean_{tag}", tag=f"ln_mean_{tag}")
    var = pool.tile([P, 1], F32, name=f"ln_var_{tag}", tag="ln_var")
    m2 = pool.tile([P, 1], F32, name=f"ln_m2_{tag}", tag="ln_m2")
    std = pool.tile([P, 1], F32, name=f"ln_std_{tag}", tag="ln_std")
    rstd = pool.tile([P, 1], F32, name=f"ln_rstd_{tag}", tag=f"ln_rstd_{tag}")
    if s1b is None:
        nc.vector.tensor_scalar_mul(mean[:, :], s1[:, :], 1.0 / C)
    else:
        nc.vector.tensor_scalar(
            mean[:, :], s1[:, :], s1b[:, :], 1.0 / C, OP.add, OP.mult
        )
    nc.vector.tensor_mul(m2[:, :], mean[:, :], mean[:, :])
    nc.vector.tensor_scalar(var[:, :], ssq[:, :], 1.0 / C, EPS, OP.mult, OP.add)
    nc.vector.tensor_sub(var[:, :], var[:, :], m2[:, :])
    nc.scalar.activation(std[:, :], var[:, :], AF.Sqrt)
    nc.vector.reciprocal(rstd[:, :], std[:, :])
    nc.vector.tensor_scalar(
        z_bf, src, mean[:, :], rstd[:, :], OP.subtract, OP.mult
    )


def _build(level=5, reps=1, sim=False):
    if (level, reps, sim) in _CACHE:
        return _CACHE[(level, reps, sim)]

    nc = bacc.Bacc(
        "TRN2", target_bir_lowering=False, debug=False,
        num_devices=1 if sim else 8,
    )

    # ---- kernel I/O (per core) ----
    x_in = nc.dram_tensor("x_c", [NBQ, P, C], F32, kind="ExternalInput").ap()
    wqkv_in = nc.dram_tensor("wqkv", [CH, P, 3 * C], BF16, kind="ExternalInput").ap()
    wo_in = nc.dram_tensor("wo", [CH, P, C], BF16, kind="ExternalInput").ap()
    wfc_in = nc.dram_tensor("wfc", [CH, P, FF], BF16, kind="ExternalInput").ap()
    wproj_in = nc.dram_tensor("wproj", [FCH, P, C], BF16, kind="ExternalInput").ap()
    bqk_in = nc.dram_tensor("bqk", [2 * CH, P], F32, kind="ExternalInput").ap()
    bv_in = nc.dram_tensor("bv", [1, C], BF16, kind="ExternalInput").ap()
    bo_in = nc.dram_tensor("bo_r", [1, C], BF16, kind="ExternalInput").ap()
    bfc_in = nc.dram_tensor("bfc_r", [FCH, P], F32, kind="ExternalInput").ap()
    bproj_in = nc.dram_tensor("bproj_r", [1, C], BF16, kind="ExternalInput").ap()
    ident_in = nc.dram_tensor("ident", [P, P], BF16, kind="ExternalInput").ap()
    ones_in = nc.dram_tensor("ones_r", [1, P], BF16, kind="ExternalInput").ap()
    # per-rank causal masks, duplicated along a middle axis of 2 so one
    # Pool-engine multiply covers both heads of a head pair
    masks_in = nc.dram_tensor("masks", [NR, P, 2, P], BF16, kind="ExternalInput").ap()
    out_dram = nc.dram_tensor("out_c", [NBQ, P, C], F32, kind="ExternalOutput").ap()

    KCOLS = CH * 512          # 4096 bf16 cols for K^T in the AG payload
    VCOLS = NBQ * (C + H)     # 4*1040 cols for aug-V in the AG payload
    KH = KCOLS // 2           # half-K AG payload (head-pair chunks 0-3 / 4-7)
    VH = VCOLS // 2           # half-V AG payload (token blocks 0-1 / 2-3)

    with tile.TileContext(nc) as tc:
        dramp = tc.alloc_tile_pool(name="dram", bufs=1, space="DRAM")
        rep_io = [
            dramp.tile([NBQ, P, C], F32, name=f"rep_io_{i}")
            for i in range(reps - 1)
        ]

        for rep in range(reps):
            sfx = f"_{rep}" if reps > 1 else ""
            x_src = x_in if rep == 0 else rep_io[rep - 1]
            out_tgt = out_dram if rep == reps - 1 else rep_io[rep]
            kvin_k = [
                dramp.tile([P, KH], BF16, name=f"kvink{sfx}_h{h}") for h in range(2)
            ]
            kvout_k = [
                dramp.tile([NR, P, KH], BF16, name=f"kvoutk{sfx}_h{h}")
                for h in range(2)
            ]
            kvin_v = [
                dramp.tile([P, VH], BF16, name=f"kvinv{sfx}_h{h}") for h in range(2)
            ]
            kvout_v = [
                dramp.tile([NR, P, VH], BF16, name=f"kvoutv{sfx}_h{h}")
                for h in range(2)
            ]
            # ---------------- persistent SBUF ----------------
            persist = tc.alloc_tile_pool(name=f"persist{sfx}", bufs=1, side="left")
            ident_sb = persist.tile([P, P], BF16, name="ident_sb")
            ones_sb = persist.tile([1, P], BF16, name="ones_sb")
            masks_sb = persist.tile([P, NR, 2, P], BF16, name="masks_sb")
            bqk_sb = persist.tile([P, 2 * CH], F32, name="bqk_sb")
            bv_sb = persist.tile([1, C], BF16, name="bv_sb")
            bo_sb = persist.tile([1, C], BF16, name="bo_sb")
            bfc_sb = persist.tile([P, FCH], F32, name="bfc_sb")
            bproj_sb = persist.tile([1, C], BF16, name="bproj_sb")
            qT = persist.tile([P, CH, 512], BF16, name="qT")
            yT = persist.tile([P, CH, 512], BF16, name="yT")
            probe_sb = (
                persist.tile([P, C], F32, name="probe_sb") if level < 4 else None
            )

            # K^T gathered from all 4 ranks: [d-part, head-pair chunk, rank, tok]
            attnspan = tc.alloc_tile_pool(name=f"attnspan{sfx}", bufs=1, side="left")
            kfull = attnspan.tile([P, CH, NR, 512], BF16, name="kfull")
            vfull = attnspan.tile([P, NR, NBQ, C + H], BF16, name="vfull")

            # ---------------- phase 0: LN1 + QKV + AllGather ----------------
            ph0 = tc.alloc_tile_pool(name=f"ph0{sfx}", bufs=1, side="left")
            ph0w = tc.alloc_tile_pool(name=f"ph0w{sfx}", bufs=2, side="left")
            wqkv_sb = ph0.tile([P, CH, 3 * C], BF16, name="wqkv_sb")
            hT = ph0.tile([P, CH, 512], BF16, name="hT")
            kTc = ph0.tile([P, CH, 512], BF16, name="kTc")
            vc = ph0.tile([P, NBQ, C + H], BF16, name="vc")

            psQK = tc.alloc_tile_pool(name=f"psQK{sfx}", bufs=4, space="PSUM")
            psV = tc.alloc_tile_pool(name=f"psV{sfx}", bufs=2, space="PSUM")
            psT = tc.alloc_tile_pool(name=f"psT{sfx}", bufs=2, space="PSUM")

            # prefetch the sqrt act-table set (covers Copy/Square/Sqrt for
            # the whole LN) while the x DMA is still in flight
            dumt = persist.tile([1, 2], F32, name="dumt")
            if level < 5 and rep > 0:
                # bisection-only: dumt is loaded from the rep-chained input,
                # and every input-sourced DMA target gets a tiny WAW write
                # from it, so tile lifetimes are ordered across reps
                nc.sync.dma_start(dumt[0:1, 0:2], x_src[0][0:1, 0:2])
                chain_slices = [
                    ident_sb[0:1, 0:1],
                    ones_sb[0:1, 0:1],
                    bqk_sb[0:1, 0:1],
                    bv_sb[0:1, 0:1],
                    bo_sb[0:1, 0:1],
                    bfc_sb[0:1, 0:1],
                    bproj_sb[0:1, 0:1],
                ]
                chain_slices += [
                    masks_sb[0:1, rk, 0, 0:1] for rk in range(NR)
                ]
                chain_slices += [
                    wqkv_sb[0:1, c, third * C:third * C + 1]
                    for c in range(CH)
                    for third in range(3)
                ]
                for wt in chain_slices:
                    nc.vector.tensor_copy(wt, dumt[0:1, 0:1])
            else:
                nc.vector.memset(dumt[:, :], 1.0)
            nc.scalar.activation(dumt[0:1, 1:2], dumt[0:1, 0:1], AF.Sqrt)
            nc.sync.dma_start(ident_sb[:, :], ident_in)
            xts, stats = [], []
            for bq in range(NBQ):
                xt = ph0w.tile([P, C], F32, name="xt", tag="xt", bufs=4)
                nc.sync.dma_start(xt[:, :], x_src[bq])
                xts.append(xt)
                stats.append(_ln_stats(nc, ph0w, xt[:, :], f"l1_{bq}"))
                zbf = ph0w.tile([P, C], BF16, name="zbf", tag="zbf")
                with tc.high_priority():
                    _ln_finalize(
                        nc, ph0w, xts[bq][:, :], zbf[:, :], *stats[bq], f"l1_{bq}"
                    )
                for cg in range(CH // 4):
                    pt = psT.tile([P, 4, P], BF16, name="pt", tag="pt")
                    for cc in range(4):
                        c = cg * 4 + cc
                        nc.tensor.transpose(
                            pt[:, cc, :], zbf[:, c * P:(c + 1) * P], ident_sb[:, :]
                        )
                    nc.vector.tensor_copy(
                        hT[:, cg * 4:(cg + 1) * 4, bq * P:(bq + 1) * P], pt[:, :, :]
                    )

            # weight DMA in K, V, Q column order so the K projection (which
            # feeds the first AllGather) is never waiting on V/Q columns
            for c in range(CH):
                nc.sync.dma_start(wqkv_sb[:, c, C:2 * C], wqkv_in[c][:, C:2 * C])
            nc.sync.dma_start(bqk_sb[:, :], bqk_in.rearrange("a p -> p a"))
            nc.sync.dma_start(ones_sb[:, :], ones_in)
            for rk in range(NR):
                nc.sync.dma_start(
                    masks_sb[:, rk, :, :].rearrange("p s q -> p (s q)"),
                    masks_in[rk].rearrange("p s q -> p (s q)"),
                )
            nc.sync.dma_start(bv_sb[:, :], bv_in)
            for c in range(CH):
                nc.sync.dma_start(
                    wqkv_sb[:, c, 2 * C:3 * C], wqkv_in[c][:, 2 * C:3 * C]
                )
            for c in range(CH):
                nc.sync.dma_start(wqkv_sb[:, c, 0:C], wqkv_in[c][:, 0:C])
            nc.sync.dma_start(bo_sb[:, :], bo_in)
            nc.sync.dma_start(bfc_sb[:, :], bfc_in.rearrange("a p -> p a"))
            nc.sync.dma_start(bproj_sb[:, :], bproj_in)

            groups = [[0, 1, 2, 3], [4, 5, 6, 7]]

            def _ag(kvin, kvout, tag):
                if sim:
                    for r in range(NR):
                        nc.sync.dma_start(kvout[r], kvin[:, :])
                else:
                    nc.gpsimd.collective_compute(
                        "AllGather", OP.bypass, replica_groups=groups,
                        ins=[kvin.opt()], outs=[kvout.opt()],
                    )

            # Q^T and K^T: [feat, tok] via lhsT=W chunk, rhs=h^T.
            # K^T first so the AllGathers can launch while V/Q^T compute.
            def _qk_tile(ft):
                ps = psQK.tile([P, 512], F32, name="ps_qk", tag="ps_qk")
                for c in range(CH):
                    nc.tensor.matmul(
                        ps[:, :],
                        wqkv_sb[:, c, ft * P:(ft + 1) * P],
                        hT[:, c, :],
                        start=(c == 0),
                        stop=(c == CH - 1),
                    )
                dest = qT[:, ft, :] if ft < CH else kTc[:, ft - CH, :]
                nc.vector.tensor_scalar_add(dest, ps[:, :], bqk_sb[:, ft:ft + 1])

            # K in two halves. Collective launch order is K0, V0, V1, K1:
            # attention's first consumers are K half 0 then V (PV of block
            # 0), while K half 1 is not read until head pair 4 — so the K1
            # AllGather is deferred behind both V halves (collectives on a
            # replica group serialize in launch order).
            for h in range(2):
                for ft in range(CH + 4 * h, CH + 4 * (h + 1)):
                    _qk_tile(ft)
                if level >= 2:
                    nc.sync.dma_start(
                        kvin_k[h][:, :],
                        kTc[:, 4 * h:4 * (h + 1), :].rearrange("p c t -> p (c t)"),
                    )
                    _ag(kvin_k[h], kvout_k[h], f"k{h}")

            # V in [tok, feat] layout with a ones column appended per head
            # (col h*65+64) so PV also accumulates the softmax denominator.
            # Bias enters PSUM first so nothing serializes after the c loop.
            for bq in range(NBQ):
                for fb in range(2):
                    ps = psV.tile([P, 512], F32, name="ps_v", tag="ps_v")
                    nc.tensor.matmul(
                        ps[:, :],
                        ones_sb[0:1, 0:P],
                        bv_sb[0:1, fb * 512:(fb + 1) * 512],
                        start=True,
                        stop=False,
                    )
                    for c in range(CH):
                        nc.tensor.matmul(
                            ps[:, :],
                            hT[:, c, bq * P:(bq + 1) * P],
                            wqkv_sb[:, c, 2 * C + fb * 512:2 * C + (fb + 1) * 512],
                            start=False,
                            stop=(c == CH - 1),
                        )
                    dst = vc[:, bq, fb * 8 * 65:(fb + 1) * 8 * 65]
                    dst = dst.rearrange("p (h x) -> p h x", x=65)[:, :, 0:64]
                    nc.vector.tensor_copy(dst, ps.rearrange("p (h x) -> p h x", x=64))
                ones_lane = vc.rearrange("p b (h x) -> p b h x", x=65)[
                    :, bq:bq + 1, :, 64:65
                ]
                nc.vector.memset(ones_lane, 1.0)
                if level >= 2:
                    h = bq // 2
                    nc.sync.dma_start(
                        kvin_v[h][:, (bq % 2) * (C + H):(bq % 2 + 1) * (C + H)],
                        vc[:, bq, :],
                    )
                    if bq % 2 == 1:
                        _ag(kvin_v[h], kvout_v[h], f"v{h}")

            for ft in range(CH):
                _qk_tile(ft)
            if level >= 2:
                # readback ordered by when attention needs it: K half 0
                # (head pairs 0-3), V half 0 (token blocks 0-1), V half 1,
                # K half 1 — interleaved per rank
                for r in range(NR):
                    nc.sync.dma_start(
                        kfull[:, 0:4, r, :],
                        kvout_k[0][r].rearrange("p (c t) -> p c t", t=512),
                    )
                for r in range(NR):
                    nc.sync.dma_start(
                        vfull[:, r, 0:2, :],
                        kvout_v[0][r].rearrange("p (b f) -> p b f", f=C + H),
                    )
                for r in range(NR):
                    nc.sync.dma_start(
                        vfull[:, r, 2:4, :],
                        kvout_v[1][r].rearrange("p (b f) -> p b f", f=C + H),
                    )
                for r in range(NR):
                    nc.sync.dma_start(
                        kfull[:, 4:8, r, :],
                        kvout_k[1][r].rearrange("p (c t) -> p c t", t=512),
                    )

            if level == 1:
                # bisection probes: chain the dead-end K/V tensors into the
                # output path before their pool is released
                nc.vector.tensor_copy(probe_sb[:, 0:512], kTc[:, 0, :])
                nc.vector.tensor_copy(probe_sb[:, 512:], vc[:, 0, 0:512])

            psT.release()
            psV.release()
            psQK.release()
            ph0w.release()
            ph0.release()

            # ---------------- attention ----------------
            x2pool = tc.alloc_tile_pool(name=f"x2pool{sfx}", bufs=1, side="right")
            x2 = x2pool.tile([P, NBQ, C], F32, name="x2")

            wfcpool = tc.alloc_tile_pool(name=f"wfcpool{sfx}", bufs=1, side="right")
            wfc_sb = wfcpool.tile([P, CH, FF], BF16, name="wfc_sb")
            if level >= 5:
                for c in range(CH):
                    for q in range(4):
                        nc.sync.dma_start(
                            wfc_sb[:, c, q * FF // 4:(q + 1) * FF // 4],
                            wfc_in[c][:, q * FF // 4:(q + 1) * FF // 4],
                        )

            wospan = tc.alloc_tile_pool(name=f"wospan{sfx}", bufs=1, side="right")
            wo_sb = wospan.tile([P, CH, C], BF16, name="wo_sb")
            if level >= 4:
                if level < 5 and rep > 0:
                    for c in range(CH):
                        nc.vector.tensor_copy(
                            wo_sb[0:1, c, 0:1], qT[0:1, 0, 0:1]
                        )
                for c in range(CH):
                    nc.sync.dma_start(wo_sb[:, c, :], wo_in[c])

            att = tc.alloc_tile_pool(name=f"att{sfx}", bufs=1, side="right")
            psS = tc.alloc_tile_pool(name=f"psS{sfx}", bufs=3, space="PSUM")
            psY = tc.alloc_tile_pool(name=f"psY{sfx}", bufs=1, space="PSUM")

            # Software-pipelined attention: PE is in-order, so a PV matmul
            # that waits on the exp/mask chain of its own tile would also
            # stall the next tile's QK matmuls queued behind it. Issuing
            # QK(i+LA) before PV(i) keeps PE a tile ahead of the ACT chain.
            LA = 2
            TILES = [(bk, rk) for bk in range(NBQ) for rk in range(NR)]
            for hp in range(HP if level >= 3 else 0):
                psy = [
                    psY.tile([65, 512], F32, name=f"psy{sub}_{hp}", tag=f"psy{sub}")
                    for sub in range(2)
                ]
                pend = []
                for i in range(len(TILES) + LA):
                    if i < len(TILES):
                        bk, rk = TILES[i]
                        qo = bk * P
                        # both heads of the pair score into one 2-bank
                        # psum tile; one Exp covers both
                        pss = psS.tile([P, 2, 512], F32, name="pss", tag="pss")
                        for sub in range(2):
                            po = sub * 64
                            nc.tensor.matmul(
                                pss[:, sub, qo:],
                                kfull[po:po + 64, hp, rk, bk * P:(bk + 1) * P],
                                qT[po:po + 64, hp, qo:],
                                start=True,
                                stop=True,
                                tile_position=(po, 0),
                            )
                        pbf = att.tile(
                            [P, 2, 512], BF16, name="pbf", tag="pbf", bufs=4
                        )
                        nc.scalar.activation(
                            pbf[:, :, qo:], pss[:, :, qo:], AF.Exp,
                            scale=1.0 / 8.0,
                        )
                        # causal mask on the diagonal block, fused over both
                        # heads of the pair
                        nc.vector.tensor_mul(
                            pbf[:, :, qo:qo + P], pbf[:, :, qo:qo + P],
                            masks_sb[:, rk, :, :],
                        )
                        pend.append((bk, rk, pbf))
                    if i >= LA:
                        j = i - LA
                        bk, rk, pbf = pend.pop(0)
                        qo = bk * P
                        for sub in range(2):
                            h = 2 * hp + sub
                            nc.tensor.matmul(
                                psy[sub][:, qo:],
                                vfull[:, rk, bk, h * 65:(h + 1) * 65],
                                pbf[:, sub, qo:],
                                start=(j == 0),
                                stop=(j == len(TILES) - 1),
                                skip_group_check=True,
                            )
                for sub in range(2):
                    po = sub * 64
                    recip = att.tile([1, 512], BF16, name="recip", tag="recip", bufs=2)
                    with nc.allow_low_precision(reason="softmax normalizer"):
                        nc.vector.reciprocal(recip[:, :], psy[sub][64:65, :])
                    bcast = att.tile([64, 512], BF16, name="bcast", tag="bcast", bufs=2)
                    nc.gpsimd.partition_broadcast(bcast[:, :], recip[0:1, :])
                    nc.vector.tensor_mul(
                        yT[po:po + 64, hp, :], psy[sub][0:64, :], bcast[:, :]
                    )

            psY.release()
            psS.release()
            att.release()
            if level >= 3:
                attnspan.release()

            # ---------------- attention out-proj + residual ----------------
            wpool = tc.alloc_tile_pool(name=f"wpool{sfx}", bufs=2, side="right")
            psW = tc.alloc_tile_pool(name=f"psW{sfx}", bufs=3, space="PSUM")
            stats2 = []
            for bq in range(NBQ if level >= 4 else 0):
                xw = wpool.tile([P, C], F32, name="xw", tag="xw")
                nc.sync.dma_start(xw[:, :], x_src[bq])
                for cb in range(2):
                    ps = psW.tile([P, 512], F32, name="ps_w", tag="ps_w")
                    nc.tensor.matmul(
                        ps[:, :],
                        ones_sb[0:1, 0:P],
                        bo_sb[0:1, cb * 512:(cb + 1) * 512],
                        start=True,
                        stop=False,
                    )
                    for c in range(CH):
                        nc.tensor.matmul(
                            ps[:, :],
                            yT[:, c, bq * P:(bq + 1) * P],
                            wo_sb[:, c, cb * 512:(cb + 1) * 512],
                            start=False,
                            stop=(c == CH - 1),
                        )
                    nc.vector.tensor_add(
                        x2[:, bq, cb * 512:(cb + 1) * 512], ps[:, :],
                        xw[:, cb * 512:(cb + 1) * 512],
                    )
                # LN2 statistics immediately after each block's residual is
                # ready, so the MLP can start without a stats bubble
                if level >= 5:
                    stats2.append(_ln_stats(nc, persist, x2[:, bq, :], f"l2_{bq}"))
            psW.release()
            wpool.release()
            wospan.release()

            # ---------------- MLP ----------------
            mpool = tc.alloc_tile_pool(name=f"mpool{sfx}", bufs=1, side="right")
            mw = tc.alloc_tile_pool(name=f"mw{sfx}", bufs=2, side="right")
            h2T = mpool.tile([P, CH, 512], BF16, name="h2T")
            gT = mpool.tile([P, FCH, 512], BF16, name="gT")

            psT2 = tc.alloc_tile_pool(name=f"psT2{sfx}", bufs=4, space="PSUM")
            for bq in range(NBQ if level >= 5 else 0):
                z2 = mw.tile([P, C], BF16, name="z2", tag="z2")
                _ln_finalize(
                    nc, mw, x2[:, bq, :], z2[:, :], *stats2[bq], f"l2_{bq}"
                )
                for cg in range(CH // 4):
                    pt2 = psT2.tile([P, 4, P], BF16, name="pt2", tag="pt2")
                    for cc in range(4):
                        c = cg * 4 + cc
                        nc.tensor.transpose(
                            pt2[:, cc, :], z2[:, c * P:(c + 1) * P], ident_sb[:, :]
                        )
                    nc.vector.tensor_copy(
                        h2T[:, cg * 4:(cg + 1) * 4, bq * P:(bq + 1) * P], pt2[:, :, :]
                    )
            psT2.release()

            psFC = tc.alloc_tile_pool(name=f"psFC{sfx}", bufs=3, space="PSUM")
            for ft in range(FCH if level >= 5 else 0):
                ps = psFC.tile([P, 512], F32, name="ps_fc", tag="ps_fc")
                for c in range(CH):
                    nc.tensor.matmul(
                        ps[:, :],
                        wfc_sb[:, c, ft * P:(ft + 1) * P],
                        h2T[:, c, :],
                        start=(c == 0),
                        stop=(c == CH - 1),
                    )
                nc.scalar.activation(
                    gT[:, ft, :], ps[:, :], AF.Gelu, bias=bfc_sb[:, ft:ft + 1]
                )
            psFC.release()

            psPJ = tc.alloc_tile_pool(name=f"psPJ{sfx}", bufs=1, space="PSUM")
            pres = [
                psPJ.tile([P, 512], F32, name=f"pres_{i}", tag=f"pres_{i}")
                for i in range(2 * NBQ)
            ] if level >= 5 else []
            # bias lands in PSUM first; the residual add + store then follow
            # the last fc accumulation with no serial bias matmul in the tail
            for bq in range(NBQ if level >= 5 else 0):
                for cb in range(2):
                    nc.tensor.matmul(
                        pres[bq * 2 + cb][:, :],
                        ones_sb[0:1, 0:P],
                        bproj_sb[0:1, cb * 512:(cb + 1) * 512],
                        start=True,
                        stop=False,
                    )
            # all blocks share the streamed wp tiles for fc < TAIL; the last
            # TAIL chunks run block-major so each block's residual add and
            # store overlap the remaining blocks' matmuls
            TAIL = 4
            wps = {}
            for fc in range(FCH if level >= 5 else 0):
                wp = mw.tile([P, C], BF16, name="wp", tag="wp", bufs=6)
                nc.sync.dma_start(wp[:, :], wproj_in[fc])
                if fc >= FCH - TAIL:
                    wps[fc] = wp
                    continue
                for bq in range(NBQ):
                    for cb in range(2):
                        nc.tensor.matmul(
                            pres[bq * 2 + cb][:, :],
                            gT[:, fc, bq * P:(bq + 1) * P],
                            wp[:, cb * 512:(cb + 1) * 512],
                            start=False,
                            stop=False,
                        )
            for bq in range(NBQ if level >= 5 else 0):
                for fc in range(FCH - TAIL, FCH):
                    for cb in range(2):
                        nc.tensor.matmul(
                            pres[bq * 2 + cb][:, :],
                            gT[:, fc, bq * P:(bq + 1) * P],
                            wps[fc][:, cb * 512:(cb + 1) * 512],
                            start=False,
                            stop=(fc == FCH - 1),
                        )
                outt = mw.tile([P, C], F32, name="outt", tag="outt")
                for cb in range(2):
                    nc.vector.tensor_add(
                        outt[:, cb * 512:(cb + 1) * 512],
                        pres[bq * 2 + cb][:, :],
                        x2[:, bq, cb * 512:(cb + 1) * 512],
                    )
                    nc.sync.dma_start(
                        out_tgt[bq][:, cb * 512:(cb + 1) * 512],
                        outt[:, cb * 512:(cb + 1) * 512],
                    )
            if level < 5:
                # bisection-only path: the output must DEPEND on the last
                # phase that ran, so reps serialize and tile lifetimes have
                # a happens-before across reps
                if level >= 4:
                    for bq in range(NBQ):
                        nc.sync.dma_start(out_tgt[bq], x2[:, bq, :])
                else:
                    # probe every dead-end tensor of the level so all work
                    # of rep k happens-before rep k+1 (copies chain WAW)
                    if level == 2:
                        nc.vector.tensor_copy(
                            probe_sb[:, 0:512], kfull[:, 0, 0, :]
                        )
                        nc.vector.tensor_copy(
                            probe_sb[:, 512:], vfull[:, 0, 0, 0:512]
                        )
                    if level <= 2:
                        nc.vector.tensor_copy(probe_sb[:, 0:512], qT[:, 0, :])
                    if level == 3:
                        nc.vector.tensor_copy(
                            probe_sb[:, :],
                            yT[:, 0:2, :].rearrange("p c t -> p (c t)"),
                        )
                    for bq in range(NBQ):
                        nc.sync.dma_start(out_tgt[bq], probe_sb[:, :])
                    if level < 3:
                        attnspan.release()
            psPJ.release()
            mw.release()
            mpool.release()
            wfcpool.release()
            x2pool.release()
            persist.release()
        dramp.release()

    if not sim:
        nc.compile()
    _CACHE[(level, reps, sim)] = nc
    return nc


def prepare_in_maps(inputs):
    """Host-side prep: fold LN, cast/shard weights, build per-core input maps."""
    x = np.asarray(inputs["x"], dtype=np.float32)
    ln1_w = np.asarray(inputs["ln1_w"], dtype=np.float32)
    ln1_b = np.asarray(inputs["ln1_b"], dtype=np.float32)
    Wqkv = np.asarray(inputs["Wqkv"], dtype=np.float32)
    bqkv = np.asarray(inputs["bqkv"], dtype=np.float32)
    Wo = np.asarray(inputs["Wo"], dtype=np.float32)
    bo = np.asarray(inputs["bo"], dtype=np.float32)
    ln2_w = np.asarray(inputs["ln2_w"], dtype=np.float32)
    ln2_b = np.asarray(inputs["ln2_b"], dtype=np.float32)
    Wfc = np.asarray(inputs["Wfc"], dtype=np.float32)
    bfc = np.asarray(inputs["bfc"], dtype=np.float32)
    Wproj = np.asarray(inputs["Wproj"], dtype=np.float32)
    bproj = np.asarray(inputs["bproj"], dtype=np.float32)

    # Fold LN affine params into the downstream matmuls.
    Wqkv_f = ln1_w[:, None] * Wqkv
    bqkv_f = bqkv + ln1_b @ Wqkv
    Wfc_f = ln2_w[:, None] * Wfc
    bfc_f = bfc + ln2_b @ Wfc

    wqkv_h = np.ascontiguousarray(Wqkv_f.astype(BF16_NP).reshape(CH, P, 3 * C))
    wo_h = np.ascontiguousarray(Wo.astype(BF16_NP).reshape(CH, P, C))
    wfc_h = np.ascontiguousarray(Wfc_f.astype(BF16_NP).reshape(CH, P, FF))
    wproj_h = np.ascontiguousarray(Wproj.astype(BF16_NP).reshape(FCH, P, C))
    bqk_h = np.ascontiguousarray(bqkv_f[: 2 * C].reshape(2 * CH, P))
    bv_h = np.ascontiguousarray(bqkv_f[2 * C:].astype(BF16_NP).reshape(1, C))
    bo_h = bo.astype(BF16_NP).reshape(1, C)
    bfc_h = np.ascontiguousarray(bfc_f.reshape(FCH, P))
    bproj_h = bproj.astype(BF16_NP).reshape(1, C)
    ident_h = np.eye(P, dtype=BF16_NP)
    ones_h = np.ones((1, P), BF16_NP)
    kk = np.arange(P)[:, None]
    qq = np.arange(P)[None, :]
    tri_incl = (kk <= qq).astype(BF16_NP)
    tri_strict = (kk < qq).astype(BF16_NP)

    in_maps = []
    for core in range(8):
        b, s = divmod(core, 4)
        x_c = np.ascontiguousarray(x[b, s::4, :]).reshape(NBQ, P, C)
        masks_h = np.stack(
            [
                np.stack([m, m])
                for m in (
                    tri_incl if rk <= s else tri_strict for rk in range(NR)
                )
            ]
        ).transpose(0, 2, 1, 3)  # [NR, P(k), 2, P(q)]
        in_maps.append(
            {
                "x_c": x_c,
                "wqkv": wqkv_h,
                "wo": wo_h,
                "wfc": wfc_h,
                "wproj": wproj_h,
                "bqk": bqk_h,
                "bv": bv_h,
                "bo_r": bo_h,
                "bfc_r": bfc_h,
                "bproj_r": bproj_h,
                "ident": ident_h,
                "ones_r": ones_h,
                "masks": masks_h,
            }
        )

    return in_maps


def assemble_output(results):
    out = np.empty((B, T, C), np.float32)
    for core in range(8):
        b, s = divmod(core, 4)
        out[b, s::4, :] = results[core]["out_c"].reshape(NR * P, C)
    return out


def kernel(**inputs):
    global LAST_RESULTS
    in_maps = prepare_in_maps(inputs)
    nc = _build(LEVEL, REPS)
    res = run_bass_kernel_spmd(
        nc, in_maps, core_ids=list(range(8)), trace=TRACE
    )
    LAST_RESULTS = res
    return assemble_output(res.results)

